# revision 3
# baseline (speedup 1.0000x reference)
"""Kalman filter (state=16, obs=96, T=8192) on 8 Trainium2 NeuronCores.

Math: with isotropic A=alpha*I, Q=q*I, R=r*I, P0=p0*I the whole Riccati
trajectory is diagonal in the fixed orthonormal eigenbasis U of C^T C
(SVD C = Z diag(sig) U^T).  The filter reduces to 16 independent scalar
recurrences z_t = a_t * z_{t-1} + g_t * (Z^T y_t), x_t = U z_t, with
a_t, g_t from a scalar per-mode Riccati recursion (y-independent, host
precomputed in fp64).  The y-dependent work runs on device: T is split
into 8 chunks (one per core); each core does matmul Z^T @ y^T, an
elementwise multiply, a hardware prefix scan (tensor_tensor_scan), and
matmul U @ z.  The cross-chunk carry is an affine diagonal map fixed up
with host-precomputed per-chunk prefix products.
"""

import numpy as np

STATE = 16
OBS = 96
T = 8192
N_CORES = 8
L = T // N_CORES  # 1024 steps per core

_COMPILED = {}


def _build_nc():
    import concourse.tile as tile
    from concourse import bacc, mybir

    f32 = mybir.dt.float32
    nc = bacc.Bacc("TRN2", target_bir_lowering=False, debug=False,
                   num_devices=N_CORES)

    yT_d = nc.dram_tensor("yT", [OBS, L], f32, kind="ExternalInput")
    aT_d = nc.dram_tensor("aT", [STATE, L], f32, kind="ExternalInput")
    gT_d = nc.dram_tensor("gT", [STATE, L], f32, kind="ExternalInput")
    Z_d = nc.dram_tensor("Z", [OBS, STATE], f32, kind="ExternalInput")
    UT_d = nc.dram_tensor("UT", [STATE, STATE], f32, kind="ExternalInput")
    xT_d = nc.dram_tensor("xT", [STATE, L], f32, kind="ExternalOutput")

    BLK = 512  # PSUM bank = 512 fp32 per partition
    NBLK = L // BLK

    with tile.TileContext(nc) as tc:
        with (
            tc.tile_pool(name="pool", bufs=1) as pool,
            tc.tile_pool(name="psum", bufs=2, space="PSUM") as psum,
        ):
            zt = pool.tile([OBS, STATE], f32)
            nc.sync.dma_start(zt[:], Z_d[:, :])
            ut = pool.tile([STATE, STATE], f32)
            nc.sync.dma_start(ut[:], UT_d[:, :])
            at = pool.tile([STATE, L], f32)
            nc.sync.dma_start(at[:], aT_d[:, :])
            gt = pool.tile([STATE, L], f32)
            nc.sync.dma_start(gt[:], gT_d[:, :])
            yt = pool.tile([OBS, L], f32)
            nc.sync.dma_start(yt[:], yT_d[:, :])

            beta = pool.tile([STATE, L], f32)
            for b in range(NBLK):
                sl = slice(b * BLK, (b + 1) * BLK)
                wp = psum.tile([STATE, BLK], f32, tag="wp")
                nc.tensor.matmul(wp[:], zt[:], yt[:, sl], start=True, stop=True)
                nc.vector.tensor_mul(beta[:, sl], gt[:, sl], wp[:])

            zloc = pool.tile([STATE, L], f32)
            nc.vector.tensor_tensor_scan(
                zloc[:], at[:], beta[:], 0.0,
                op0=mybir.AluOpType.mult, op1=mybir.AluOpType.add,
            )

            xout = pool.tile([STATE, L], f32)
            for b in range(NBLK):
                sl = slice(b * BLK, (b + 1) * BLK)
                xp = psum.tile([STATE, BLK], f32, tag="xp")
                nc.tensor.matmul(xp[:], ut[:], zloc[:, sl], start=True, stop=True)
                nc.vector.tensor_copy(xout[:, sl], xp[:])
            nc.sync.dma_start(xT_d[:, :], xout[:])

    nc.compile()
    return nc


def _host_precompute(A, C, Q, R, x_init, P_init):
    """fp64 y-independent precompute: SVD of C + per-mode scalar Riccati."""
    A64 = A.astype(np.float64)
    C64 = C.astype(np.float64)
    alpha = A64[0, 0]
    q = Q.astype(np.float64)[0, 0]
    r = R.astype(np.float64)[0, 0]
    p0 = P_init.astype(np.float64)[0, 0]

    Zs, sig, UT = np.linalg.svd(C64, full_matrices=False)
    U = UT.T

    d = np.full(STATE, p0)
    a_seq = np.empty((T, STATE))
    g_seq = np.empty((T, STATE))
    for t in range(T):
        dp = alpha * alpha * d + q
        g = dp * sig / (sig * sig * dp + r)
        oneminus = 1.0 - sig * g
        a_seq[t] = alpha * oneminus
        g_seq[t] = g
        d = oneminus * dp

    # per-chunk prefix products of a (fp64)
    pi = np.empty((T, STATE))
    for c in range(N_CORES):
        acc = np.ones(STATE)
        for i in range(L):
            acc = acc * a_seq[c * L + i]
            pi[c * L + i] = acc
    Ac = pi[np.arange(1, N_CORES + 1) * L - 1]  # [8,16] total chunk products

    z0 = U.T @ x_init.astype(np.float64)
    return Zs, U, a_seq, g_seq, pi, Ac, z0


def _isotropic(M, dim):
    c = M[0, 0]
    return bool(np.abs(M - c * np.eye(dim, dtype=M.dtype)).max() <= 1e-30)


def _fallback(y_seq, A, C, Q, R, x_init, P_init):
    """General (non-isotropic) inputs: plain fp32 numpy filter."""
    f = np.float32
    A = A.astype(f); C = C.astype(f); Q = Q.astype(f); R = R.astype(f)
    x = x_init.astype(f); P = P_init.astype(f)
    I = np.eye(STATE, dtype=f)
    out = np.empty((T, STATE), f)
    for t in range(T):
        x_pred = A @ x
        P_pred = A @ P @ A.T + Q
        S = C @ P_pred @ C.T + R
        K = (P_pred @ C.T @ np.linalg.inv(S)).astype(f)
        x = x_pred + K @ (y_seq[t].astype(f) - C @ x_pred)
        P = ((I - K @ C) @ P_pred).astype(f)
        out[t] = x
    return out


def kernel(y_seq, A, C, Q, R, x_init, P_init):
    y_seq = np.asarray(y_seq)
    A = np.asarray(A); C = np.asarray(C); Q = np.asarray(Q)
    R = np.asarray(R)
    x_init = np.asarray(x_init); P_init = np.asarray(P_init)

    if not (_isotropic(A, STATE) and _isotropic(Q, STATE)
            and _isotropic(R, OBS) and _isotropic(P_init, STATE)):
        return _fallback(y_seq, A, C, Q, R, x_init, P_init)

    Zs, U, a_seq, g_seq, pi, Ac, z0 = _host_precompute(
        A, C, Q, R, x_init, P_init)

    f = np.float32
    Zf = np.ascontiguousarray(Zs, f)
    UTf = np.ascontiguousarray(U.T, f)

    if "nc" not in _COMPILED:
        _COMPILED["nc"] = _build_nc()
    nc = _COMPILED["nc"]

    in_maps = []
    for c in range(N_CORES):
        sl = slice(c * L, (c + 1) * L)
        in_maps.append({
            "yT": np.ascontiguousarray(y_seq[sl].T, f),
            "aT": np.ascontiguousarray(a_seq[sl].T, f),
            "gT": np.ascontiguousarray(g_seq[sl].T, f),
            "Z": Zf,
            "UT": UTf,
        })

    from concourse.bass_utils import run_bass_kernel_spmd
    res = run_bass_kernel_spmd(nc, in_maps, core_ids=list(range(N_CORES)))

    # host carry stitch: x_glob = x_loc + U (pi ⊙ z_entry)
    Uf = np.ascontiguousarray(U, f)
    out = np.empty((T, STATE), f)
    zle = np.empty((N_CORES, STATE), f)
    xTs = []
    for c in range(N_CORES):
        xT = res.results[c]["xT"]  # [16, L] = U @ z_loc
        xTs.append(xT)
        zle[c] = (Uf.T @ xT[:, -1]).astype(f)

    Acf = Ac.astype(f)
    pif = pi.astype(f)
    z0f = z0.astype(f)
    for c in range(N_CORES):
        # z_entry_c = sum_{j<c} (prod_{j<i<c} Ac_i) * zle_j + (prod_{i<c} Ac_i) * z0
        e = np.zeros(STATE, f)
        w = np.ones(STATE, f)
        for j in range(c - 1, -1, -1):
            e = (e + w * zle[j]).astype(f)
            w = (w * Acf[j]).astype(f)
        e = (e + w * z0f).astype(f)
        sl = slice(c * L, (c + 1) * L)
        corr = (pif[sl] * e[None, :]).astype(f) @ Uf.T
        out[sl] = xTs[c].T + corr.astype(f)

    return out


# revision 5
# speedup vs baseline: 1.3464x; 1.3464x over previous
"""Kalman filter (state=16, obs=96, T=8192) on 8 Trainium2 NeuronCores.

Math: with isotropic A=alpha*I, Q=q*I, R=r*I, P0=p0*I the whole Riccati
trajectory is diagonal in the fixed orthonormal eigenbasis U of C^T C
(SVD C = Z diag(sig) U^T).  The filter reduces to 16 independent scalar
recurrences z_t = a_t * z_{t-1} + g_t * (Z^T y_t), x_t = U z_t, with
a_t, g_t from a scalar per-mode Riccati recursion (y-independent, host
precomputed in fp64).  The y-dependent work runs on device: T is split
into 8 chunks (one per core); each core does matmul Z^T @ y^T, an
elementwise multiply, a hardware prefix scan (tensor_tensor_scan), and
matmul U @ z.  The cross-chunk carry is an affine diagonal map fixed up
with host-precomputed per-chunk prefix products.
"""

import numpy as np

STATE = 16
OBS = 96
T = 8192
N_CORES = 8
L = T // N_CORES  # 1024 steps per core

_COMPILED = {}


def _build_nc():
    import concourse.tile as tile
    from concourse import bacc, mybir

    f32 = mybir.dt.float32
    NSEG = 4
    SEG = L // NSEG
    nc = bacc.Bacc("TRN2", target_bir_lowering=False, debug=False,
                   num_devices=N_CORES)

    # consolidated inputs (DMA issue on the sequencer costs ~1.6us each):
    # yzu = [y^T | Z | UT padded to 96 rows]  [96, L+32]
    # ag  = [a^T | g^T]                       [16, 2L]
    yzu_d = nc.dram_tensor("yzu", [OBS, L + 32], f32, kind="ExternalInput")
    ag_d = nc.dram_tensor("ag", [STATE, 2 * L], f32, kind="ExternalInput")
    xT_d = nc.dram_tensor("xT", [STATE, L], f32, kind="ExternalOutput")

    with tile.TileContext(nc) as tc:
        with (
            tc.tile_pool(name="pool", bufs=1) as pool,
            tc.tile_pool(name="psum", bufs=4, space="PSUM") as psum,
        ):
            yzu = pool.tile([OBS, L + 32], f32)
            nc.sync.dma_start(yzu[:], yzu_d[:, :])
            ag = pool.tile([STATE, 2 * L], f32)
            nc.gpsimd.dma_start(ag[:], ag_d[:, :])
            zt = yzu[:, L:L + 16]
            ut = yzu[:16, L + 16:L + 32]

            xout = pool.tile([STATE, L], f32)
            zloc_prev = None
            for s in range(NSEG):
                sl = slice(s * SEG, (s + 1) * SEG)
                wp = psum.tile([STATE, SEG], f32, tag="wp", bufs=2)
                nc.tensor.matmul(wp[:], zt, yzu[:, sl], start=True, stop=True)
                beta = pool.tile([STATE, SEG], f32, tag="beta", bufs=2,
                                 name=f"beta{s}")
                nc.vector.tensor_mul(
                    beta[:], ag[:, L + s * SEG:L + (s + 1) * SEG], wp[:])
                zloc = pool.tile([STATE, SEG], f32, tag="zloc", bufs=2,
                                 name=f"zloc{s}")
                init = 0.0 if s == 0 else zloc_prev[:, SEG - 1:SEG]
                nc.vector.tensor_tensor_scan(
                    zloc[:], ag[:, sl], beta[:], init,
                    op0=mybir.AluOpType.mult, op1=mybir.AluOpType.add,
                )
                zloc_prev = zloc
                xp = psum.tile([STATE, SEG], f32, tag="xp", bufs=2)
                nc.tensor.matmul(xp[:], ut, zloc[:], start=True, stop=True)
                nc.vector.tensor_copy(xout[:, sl], xp[:])
            nc.sync.dma_start(xT_d[:, :], xout[:])

    nc.compile()
    return nc


def _host_precompute(A, C, Q, R, x_init, P_init):
    """fp64 y-independent precompute: SVD of C + per-mode scalar Riccati."""
    A64 = A.astype(np.float64)
    C64 = C.astype(np.float64)
    alpha = A64[0, 0]
    q = Q.astype(np.float64)[0, 0]
    r = R.astype(np.float64)[0, 0]
    p0 = P_init.astype(np.float64)[0, 0]

    Zs, sig, UT = np.linalg.svd(C64, full_matrices=False)
    U = UT.T

    d = np.full(STATE, p0)
    a_seq = np.empty((T, STATE))
    g_seq = np.empty((T, STATE))
    for t in range(T):
        dp = alpha * alpha * d + q
        g = dp * sig / (sig * sig * dp + r)
        oneminus = 1.0 - sig * g
        a_seq[t] = alpha * oneminus
        g_seq[t] = g
        d = oneminus * dp

    # per-chunk prefix products of a (fp64)
    pi = np.empty((T, STATE))
    for c in range(N_CORES):
        acc = np.ones(STATE)
        for i in range(L):
            acc = acc * a_seq[c * L + i]
            pi[c * L + i] = acc
    Ac = pi[np.arange(1, N_CORES + 1) * L - 1]  # [8,16] total chunk products

    z0 = U.T @ x_init.astype(np.float64)
    return Zs, U, a_seq, g_seq, pi, Ac, z0


def _isotropic(M, dim):
    c = M[0, 0]
    return bool(np.abs(M - c * np.eye(dim, dtype=M.dtype)).max() <= 1e-30)


def _fallback(y_seq, A, C, Q, R, x_init, P_init):
    """General (non-isotropic) inputs: plain fp32 numpy filter."""
    f = np.float32
    A = A.astype(f); C = C.astype(f); Q = Q.astype(f); R = R.astype(f)
    x = x_init.astype(f); P = P_init.astype(f)
    I = np.eye(STATE, dtype=f)
    out = np.empty((T, STATE), f)
    for t in range(T):
        x_pred = A @ x
        P_pred = A @ P @ A.T + Q
        S = C @ P_pred @ C.T + R
        K = (P_pred @ C.T @ np.linalg.inv(S)).astype(f)
        x = x_pred + K @ (y_seq[t].astype(f) - C @ x_pred)
        P = ((I - K @ C) @ P_pred).astype(f)
        out[t] = x
    return out


def kernel(y_seq, A, C, Q, R, x_init, P_init):
    y_seq = np.asarray(y_seq)
    A = np.asarray(A); C = np.asarray(C); Q = np.asarray(Q)
    R = np.asarray(R)
    x_init = np.asarray(x_init); P_init = np.asarray(P_init)

    if not (_isotropic(A, STATE) and _isotropic(Q, STATE)
            and _isotropic(R, OBS) and _isotropic(P_init, STATE)):
        return _fallback(y_seq, A, C, Q, R, x_init, P_init)

    Zs, U, a_seq, g_seq, pi, Ac, z0 = _host_precompute(
        A, C, Q, R, x_init, P_init)

    f = np.float32
    Zf = np.ascontiguousarray(Zs, f)
    UTf = np.ascontiguousarray(U.T, f)

    if "nc" not in _COMPILED:
        _COMPILED["nc"] = _build_nc()
    nc = _COMPILED["nc"]

    UTpad = np.zeros((OBS, STATE), f)
    UTpad[:STATE, :] = UTf
    in_maps = []
    for c in range(N_CORES):
        sl = slice(c * L, (c + 1) * L)
        yzu = np.empty((OBS, L + 32), f)
        yzu[:, :L] = y_seq[sl].T
        yzu[:, L:L + 16] = Zf
        yzu[:, L + 16:] = UTpad
        ag = np.empty((STATE, 2 * L), f)
        ag[:, :L] = a_seq[sl].T
        ag[:, L:] = g_seq[sl].T
        in_maps.append({"yzu": yzu, "ag": ag})

    from concourse.bass_utils import run_bass_kernel_spmd
    res = run_bass_kernel_spmd(nc, in_maps, core_ids=list(range(N_CORES)))

    # host carry stitch: x_glob = x_loc + U (pi ⊙ z_entry)
    Uf = np.ascontiguousarray(U, f)
    out = np.empty((T, STATE), f)
    zle = np.empty((N_CORES, STATE), f)
    xTs = []
    for c in range(N_CORES):
        xT = res.results[c]["xT"]  # [16, L] = U @ z_loc
        xTs.append(xT)
        zle[c] = (Uf.T @ xT[:, -1]).astype(f)

    Acf = Ac.astype(f)
    pif = pi.astype(f)
    z0f = z0.astype(f)
    for c in range(N_CORES):
        # z_entry_c = sum_{j<c} (prod_{j<i<c} Ac_i) * zle_j + (prod_{i<c} Ac_i) * z0
        e = np.zeros(STATE, f)
        w = np.ones(STATE, f)
        for j in range(c - 1, -1, -1):
            e = (e + w * zle[j]).astype(f)
            w = (w * Acf[j]).astype(f)
        e = (e + w * z0f).astype(f)
        sl = slice(c * L, (c + 1) * L)
        corr = (pif[sl] * e[None, :]).astype(f) @ Uf.T
        out[sl] = xTs[c].T + corr.astype(f)

    return out
